# revision 16
# baseline (speedup 1.0000x reference)
"""DeepCluster vq_codebook kernel for TRN2 (8 NeuronCores, data-parallel over video batch).

Layout per core (core c of 8): videos b in [8c, 8c+8), all 6 after-frames.
Local sample s in [0,48): b_loc = s//6, a = s%6; x-shard Bt base = 2*s.
Each core computes its 48 samples' centroids (3-layer token MLP + LN),
squared distances, cluster assignment, medoids, counting-sort of medoid
indices, and gathers the medoid rows from its x-shard via indirect DMA.
Host concatenates per-core outputs and averages the loss partials.

All argmin/medoid/sort decisions run on squared distances (monotone in the
reference's sqrt distances), so the loose-precision ACT sqrt only touches
the loss value.
"""

import numpy as np

import concourse.bass as bass
import concourse.bacc as bacc
import concourse.mybir as mybir
import concourse.tile as tile
import concourse.bass_isa as bass_isa
from concourse.alu_op_type import AluOpType
from concourse.bass import IndirectOffsetOnAxis
from concourse.bass_utils import run_bass_kernel_spmd

F32 = mybir.dt.float32
BF16 = mybir.dt.bfloat16
I32 = mybir.dt.int32
U32 = mybir.dt.uint32
AF_SQRT = mybir.ActivationFunctionType.Sqrt

L_TOK = 50
BT = 768
W = 768
B = 64
AFTER_F = 6
NCORES = 8
BPC = B // NCORES       # 8 videos / core
S = BPC * AFTER_F       # 48 samples / core
BT_C = BPC * 12         # 96 Bt rows / core
LP = 98                 # tokens per sample
D1 = 392
D2 = 98
K = 49
NCH = 6
EPS_LN = 1e-5
DEBUG = False
import os as _os
STAGE = int(_os.environ.get("KSTAGE", "3"))

XL = BT_C * W           # stride of token-row axis (l) in the x shard
XB = W                  # stride of the bt axis


def _bc_free(ap, axis, n):
    """Broadcast ap along a new free axis (step 0)."""
    a = ap.unsqueeze(axis)
    shape = list(a.shape)
    shape[axis] = n
    return a.broadcast_to(shape)


def build():
    nc = bacc.Bacc(None, target_bir_lowering=False)

    x = nc.dram_tensor("x", [L_TOK, BT_C, W], F32, kind="ExternalInput")
    fc1_w = nc.dram_tensor("fc1_w", [LP, D1], F32, kind="ExternalInput")
    fc1_b = nc.dram_tensor("fc1_b", [D1], F32, kind="ExternalInput")
    ln1_w = nc.dram_tensor("ln1_w", [D1], F32, kind="ExternalInput")
    ln1_b = nc.dram_tensor("ln1_b", [D1], F32, kind="ExternalInput")
    fc2_w = nc.dram_tensor("fc2_w", [D1, D2], F32, kind="ExternalInput")
    fc2_b = nc.dram_tensor("fc2_b", [D2], F32, kind="ExternalInput")
    ln2_w = nc.dram_tensor("ln2_w", [D2], F32, kind="ExternalInput")
    ln2_b = nc.dram_tensor("ln2_b", [D2], F32, kind="ExternalInput")
    fc3_w = nc.dram_tensor("fc3_w", [D2, K], F32, kind="ExternalInput")
    fc3_b = nc.dram_tensor("fc3_b", [K], F32, kind="ExternalInput")
    ln3_w = nc.dram_tensor("ln3_w", [K], F32, kind="ExternalInput")
    ln3_b = nc.dram_tensor("ln3_b", [K], F32, kind="ExternalInput")
    out = nc.dram_tensor("out", [L_TOK, S, W], F32, kind="ExternalOutput")
    loss = nc.dram_tensor("loss", [1, 1], F32, kind="ExternalOutput")
    csb_scr = nc.dram_tensor("csb_scr", [S * LP], F32, kind="Internal")
    if DEBUG:
        d2dbg = nc.dram_tensor("d2dbg", [LP, S, K], F32, kind="ExternalOutput")
        ctdbg = nc.dram_tensor("ctdbg", [128, NCH, K, S], F32, kind="ExternalOutput")
        a1dbg = nc.dram_tensor("a1dbg", [128, NCH, D1], F32, kind="ExternalOutput")
        a2dbg = nc.dram_tensor("a2dbg", [128, NCH, D2], F32, kind="ExternalOutput")

    x_ap = x[:]

    with tile.TileContext(nc) as tc:
        with (
            tc.tile_pool(name="const", bufs=1) as cpool,
            tc.tile_pool(name="persist", bufs=1) as ppool,
            tc.tile_pool(name="data", bufs=3) as dpool,
            tc.tile_pool(name="dataT", bufs=2) as dtpool,
            tc.tile_pool(name="ctr", bufs=2) as ctpool,
            tc.tile_pool(name="act", bufs=3) as apool,
            tc.tile_pool(name="small", bufs=4) as spool,
            tc.tile_pool(name="gath", bufs=3) as gpool,
            tc.tile_pool(name="psA", bufs=2, space="PSUM") as psA,
            tc.tile_pool(name="psT", bufs=2, space="PSUM") as psT,
            tc.tile_pool(name="psB", bufs=3, space="PSUM") as psB,
            tc.tile_pool(name="psS", bufs=1, space="PSUM") as psS,
        ):
            # ---------------- one-time setup ----------------
            iot = cpool.tile([128, 128], I32)
            nc.gpsimd.iota(iot[:], [[-1, 128]], base=0, channel_multiplier=1)
            id128 = cpool.tile([128, 128], F32)
            nc.vector.tensor_scalar(id128[:], iot[:], 0, None, AluOpType.is_equal)

            ikm_i = cpool.tile([LP, K], I32)     # k - 98
            nc.gpsimd.iota(ikm_i[:], [[1, K]], base=-LP, channel_multiplier=0)
            ikm = cpool.tile([LP, K], F32)
            nc.vector.tensor_copy(ikm[:], ikm_i[:])

            ilm_i = cpool.tile([K, LP], I32)     # l - 98
            nc.gpsimd.iota(ilm_i[:], [[1, LP]], base=-LP, channel_multiplier=0)
            ilm = cpool.tile([K, LP], F32)
            nc.vector.tensor_copy(ilm[:], ilm_i[:])

            ir_i = cpool.tile([S, K], I32)       # r
            nc.gpsimd.iota(ir_i[:], [[1, K]], base=0, channel_multiplier=0)
            ir48 = cpool.tile([S, K], F32)
            nc.vector.tensor_copy(ir48[:], ir_i[:])

            st_i = cpool.tile([S, 1], I32)       # row base: 96 + 2*s
            nc.gpsimd.iota(st_i[:], [[0, 1]], base=BT_C, channel_multiplier=2)
            st48 = cpool.tile([S, 1], F32)
            nc.vector.tensor_copy(st48[:], st_i[:])

            ones1 = cpool.tile([1, 128], F32)
            nc.vector.memset(ones1[:], 1.0)
            ones98 = cpool.tile([LP, 1], F32)
            nc.vector.memset(ones98[:], 1.0)
            ones128 = cpool.tile([128, 1], F32)
            nc.vector.memset(ones128[:], 1.0)
            ones49b = cpool.tile([K, 1], BF16)
            nc.vector.memset(ones49b[:], 1.0)
            eps1 = cpool.tile([128, 1], F32)
            nc.vector.memset(eps1[:], EPS_LN)
            zeros98 = cpool.tile([LP, 1], F32)
            nc.vector.memset(zeros98[:], 0.0)

            # L1 weights augmented with identity: [fc1_w | I98]
            w1aug = cpool.tile([LP, D1 + LP], F32)
            nc.sync.dma_start(w1aug[0:LP, 0:D1], fc1_w[:])
            nc.vector.tensor_copy(w1aug[0:LP, D1:D1 + LP], id128[0:LP, 0:LP])
            # fc1_b broadcast to [128, 392]
            fb1row = spool.tile([1, D1], F32, tag="brow1")
            nc.sync.dma_start(fb1row[:], fc1_b[:].unsqueeze(0))
            psbb1 = psS.tile([128, D1], F32, tag="small")
            nc.tensor.matmul(psbb1[:], ones1[:], fb1row[:], start=True, stop=True)
            b1bc = cpool.tile([128, D1], F32)
            nc.vector.tensor_copy(b1bc[:], psbb1[:])

            # L2 weights as [98, 4, 98], rows scaled by ln1_w (ln1 folded in)
            w2 = cpool.tile([D2, 4, D2], F32)
            nc.sync.dma_start(w2[:], fc2_w[:].rearrange("(j p) f -> p j f", j=4))
            ln1w_t = cpool.tile([D2, 4], F32)
            nc.sync.dma_start(ln1w_t[:], ln1_w[:].rearrange("(j p) -> p j", j=4))
            ln1b_t = cpool.tile([D2, 4], F32)
            nc.sync.dma_start(ln1b_t[:], ln1_b[:].rearrange("(j p) -> p j", j=4))

            psb2 = psS.tile([1, D2], F32, tag="small")
            for j in range(4):
                nc.tensor.matmul(psb2[:], ln1b_t[:, j:j + 1], w2[:, j, :],
                                 start=(j == 0), stop=(j == 3))
            fcb2 = spool.tile([1, D2], F32, tag="brow")
            nc.sync.dma_start(fcb2[:], fc2_b[:].unsqueeze(0))
            b2row = cpool.tile([1, D2], F32)
            nc.vector.tensor_tensor(b2row[:], psb2[:], fcb2[:], AluOpType.add)
            for j in range(4):
                nc.vector.tensor_scalar(w2[:, j, :], w2[:, j, :],
                                        ln1w_t[:, j:j + 1], None, AluOpType.mult)
            psbc2 = psS.tile([128, D2], F32, tag="small")
            nc.tensor.matmul(psbc2[:], ones1[:], b2row[:], start=True, stop=True)
            b2bc = cpool.tile([128, D2], F32)
            nc.vector.tensor_copy(b2bc[:], psbc2[:])

            # L3 weights scaled by ln2_w; b3' = fc3_b + ln2_b @ fc3_w
            w3 = cpool.tile([D2, K], F32)
            nc.sync.dma_start(w3[:], fc3_w[:])
            ln2w_t = cpool.tile([D2, 1], F32)
            nc.sync.dma_start(ln2w_t[:], ln2_w[:].unsqueeze(1))
            ln2b_t = cpool.tile([D2, 1], F32)
            nc.sync.dma_start(ln2b_t[:], ln2_b[:].unsqueeze(1))
            psb3 = psS.tile([1, K], F32, tag="small")
            nc.tensor.matmul(psb3[:], ln2b_t[:], w3[:], start=True, stop=True)
            fcb3 = spool.tile([1, K], F32, tag="brow")
            nc.sync.dma_start(fcb3[:], fc3_b[:].unsqueeze(0))
            b3row = cpool.tile([1, K], F32)
            nc.vector.tensor_tensor(b3row[:], psb3[:], fcb3[:], AluOpType.add)
            nc.vector.tensor_scalar(w3[:], w3[:], ln2w_t[:], None, AluOpType.mult)
            psbc3 = psS.tile([128, K], F32, tag="small")
            nc.tensor.matmul(psbc3[:], ones1[:], b3row[:], start=True, stop=True)
            b3bc = cpool.tile([128, K], F32)
            nc.vector.tensor_copy(b3bc[:], psbc3[:])

            # LN3 apply tiles: -2*ln3_w, -2*ln3_b broadcast to [128, 49]
            l3row = spool.tile([1, K], F32, tag="brow")
            nc.sync.dma_start(l3row[:], ln3_w[:].unsqueeze(0))
            psw3f = psS.tile([128, K], F32, tag="small")
            nc.tensor.matmul(psw3f[:], ones1[:], l3row[:], start=True, stop=True)
            w3f = cpool.tile([128, K], F32)
            nc.vector.tensor_scalar(w3f[:], psw3f[:], -2.0, None, AluOpType.mult)
            l3brow = spool.tile([1, K], F32, tag="brow")
            nc.sync.dma_start(l3brow[:], ln3_b[:].unsqueeze(0))
            psb3f = psS.tile([128, K], F32, tag="small")
            nc.tensor.matmul(psb3f[:], ones1[:], l3brow[:], start=True, stop=True)
            b3f = cpool.tile([128, K], F32)
            nc.vector.tensor_scalar(b3f[:], psb3f[:], -2.0, None, AluOpType.mult)

            # persistent batch tiles
            d2_all = ppool.tile([LP, S, K], F32)
            scr = ppool.tile([LP, S, K], F32)
            valsT = ppool.tile([K, S, LP], F32)
            eqm = ppool.tile([K, S, LP], F32)
            cmpb = ppool.tile([K, S, LP], BF16)
            Csb = ppool.tile([1, S * LP], F32)
            C48 = ppool.tile([S, LP], F32)
            rk_t = ppool.tile([S, K, LP], F32)

            # ---------------- class-embed path ----------------
            xcls = ppool.tile([S, 2, W], F32)
            nc.sync.dma_start(
                xcls[:], x_ap[0:1, :, :].rearrange("o (s f) w -> (o s) f w", f=2))
            cls48 = ppool.tile([S, W], F32)
            nc.vector.tensor_tensor(cls48[:], xcls[:, 0, :], xcls[:, 1, :],
                                    AluOpType.add)
            nc.vector.tensor_scalar(cls48[:], cls48[:], 0.5, None, AluOpType.mult)
            nc.sync.dma_start(out[0, :, :], cls48[:])

            # ---------------- per-sample pipeline ----------------
            for s in range(S):
                dataE = dpool.tile([LP, W], F32, tag="dataE")
                for f in range(2):
                    nc.sync.dma_start(dataE[f * K:(f + 1) * K, :],
                                      x_ap[1:L_TOK, 2 * s + f, :])

                # |x_l|^2 = W * (mean^2 + var) via bn stats (2 groups of 384)
                xst = spool.tile([LP, 2, 6], F32, tag="xst")
                nc.vector.bn_stats(xst[:, 0, :], dataE[0:LP, 0:W // 2])
                nc.vector.bn_stats(xst[:, 1, :], dataE[0:LP, W // 2:W])
                xmv = spool.tile([LP, 2], F32, tag="xmv")
                nc.vector.bn_aggr(xmv[:], xst[:])
                xn2 = spool.tile([LP, 1], F32, tag="xn2")
                nc.vector.scalar_tensor_tensor(
                    xn2[:], xmv[:, 0:1], xmv[:, 0:1], xmv[:, 1:2],
                    AluOpType.mult, AluOpType.add)
                nc.vector.tensor_scalar(xn2[:], xn2[:], float(W), None,
                                        AluOpType.mult)

                dataT = dtpool.tile([128, NCH, LP], F32, tag="dataT")
                ctrTs = ctpool.tile([128, NCH, K], F32, tag="ctrTs")
                sqc = ctpool.tile([128, NCH, K], F32, tag="sqc")
                t3all = ctpool.tile([128, NCH, K], F32, tag="t3all")
                psn = psS.tile([1, K, NCH], F32, tag="small")

                for c in range(NCH):
                    # L1 (+ free data transpose and bias via the ones row)
                    psa = psA.tile([128, D1 + LP], F32, tag="psa")
                    nc.tensor.matmul(psa[:], dataE[0:LP, 128 * c:128 * (c + 1)],
                                     w1aug[:], start=True, stop=True)
                    nc.scalar.copy(dataT[:, c, :], psa[:, D1:D1 + LP])

                    t1 = apool.tile([128, D1], F32, tag="t1")
                    nc.vector.tensor_tensor(t1[:], psa[:, 0:D1], b1bc[:],
                                            AluOpType.add)
                    st6 = spool.tile([128, 6], F32, tag="st6")
                    nc.vector.bn_stats(st6[:], t1[:])
                    mv1 = spool.tile([128, 2], F32, tag="mv1")
                    nc.vector.bn_aggr(mv1[:], st6[:])
                    sd1 = spool.tile([128, 1], F32, tag="sd1")
                    nc.scalar.activation(sd1[:], mv1[:, 1:2], AF_SQRT, bias=eps1[:])
                    rs1 = spool.tile([128, 1], F32, tag="rs1")
                    nc.vector.reciprocal(rs1[:], sd1[:])
                    act1 = apool.tile([128, D1], F32, tag="act1")
                    nc.vector.tensor_scalar(act1[:], t1[:], mv1[:, 0:1],
                                            rs1[:], AluOpType.subtract,
                                            AluOpType.mult)
                    if DEBUG and s == 0:
                        nc.sync.dma_start(a1dbg[:, c, :], act1[:])

                    ps2 = psB.tile([128, D2], F32, tag="mm")
                    for j in range(4):
                        pst = psT.tile([D2, 128], F32, tag="pst")
                        nc.tensor.transpose(pst[:], act1[:, D2 * j:D2 * (j + 1)],
                                            id128[:])
                        a1t = apool.tile([D2, 128], F32, tag="a1t")
                        nc.vector.tensor_copy(a1t[:], pst[:])
                        nc.tensor.matmul(ps2[:], a1t[:], w2[:, j, :],
                                         start=(j == 0), stop=(j == 3))

                    t2 = apool.tile([128, D2], F32, tag="t2")
                    nc.vector.tensor_tensor(t2[:], ps2[:], b2bc[:], AluOpType.add)
                    st6b = spool.tile([128, 6], F32, tag="st6b")
                    nc.vector.bn_stats(st6b[:], t2[:])
                    mv2 = spool.tile([128, 2], F32, tag="mv2")
                    nc.vector.bn_aggr(mv2[:], st6b[:])
                    sd2 = spool.tile([128, 1], F32, tag="sd2")
                    nc.scalar.activation(sd2[:], mv2[:, 1:2], AF_SQRT, bias=eps1[:])
                    rs2 = spool.tile([128, 1], F32, tag="rs2")
                    nc.vector.reciprocal(rs2[:], sd2[:])
                    act2 = apool.tile([128, D2], F32, tag="act2")
                    nc.vector.tensor_scalar(act2[:], t2[:], mv2[:, 0:1],
                                            rs2[:], AluOpType.subtract,
                                            AluOpType.mult)
                    if DEBUG and s == 0:
                        nc.sync.dma_start(a2dbg[:, c, :], act2[:])

                    pst3 = psT.tile([D2, 128], F32, tag="pst")
                    nc.tensor.transpose(pst3[:], act2[:], id128[:])
                    a2t = apool.tile([D2, 128], F32, tag="a2t")
                    nc.vector.tensor_copy(a2t[:], pst3[:])
                    ps3 = psB.tile([128, K], F32, tag="mm")
                    nc.tensor.matmul(ps3[:], a2t[:], w3[:], start=True, stop=True)

                    nc.vector.tensor_tensor(t3all[:, c, :], ps3[:], b3bc[:],
                                            AluOpType.add)

                # batched LN3 over all 6 chunks (stats over the 49 clusters)
                sx6 = spool.tile([128, NCH], F32, tag="sx6")
                nc.vector.tensor_reduce(sx6[:], t3all[:], mybir.AxisListType.X,
                                        AluOpType.add)
                mu6 = spool.tile([128, NCH], F32, tag="mu6")
                nc.vector.tensor_scalar(mu6[:], sx6[:], 1.0 / K, None,
                                        AluOpType.mult)
                cen = ctpool.tile([128, NCH, K], F32, tag="cen")
                nc.vector.tensor_tensor(cen[:], t3all[:], _bc_free(mu6[:], 2, K),
                                        AluOpType.subtract)
                sq6 = ctpool.tile([128, NCH, K], F32, tag="sq6")
                nc.vector.tensor_tensor(sq6[:], cen[:], cen[:], AluOpType.mult)
                vs6 = spool.tile([128, NCH], F32, tag="vs6")
                nc.vector.tensor_reduce(vs6[:], sq6[:], mybir.AxisListType.X,
                                        AluOpType.add)
                sd6 = spool.tile([128, NCH], F32, tag="sd6")
                nc.scalar.activation(sd6[:], vs6[:], AF_SQRT, bias=eps1[:],
                                     scale=1.0 / K)
                rs6 = spool.tile([128, NCH], F32, tag="rs6")
                nc.vector.reciprocal(rs6[:], sd6[:])
                nc.vector.tensor_tensor(cen[:], cen[:], _bc_free(rs6[:], 2, K),
                                        AluOpType.mult)
                nc.vector.tensor_tensor(cen[:], cen[:], _bc_free(w3f[:], 1, NCH),
                                        AluOpType.mult)
                nc.vector.tensor_tensor(ctrTs[:], cen[:], _bc_free(b3f[:], 1, NCH),
                                        AluOpType.add)

                # |c|^2: one matmul over [k (outer), chunk (inner)] then reduce
                nc.vector.tensor_tensor(sqc[:], ctrTs[:], ctrTs[:], AluOpType.mult)
                sqc_kc = sqc[:].transpose([0, 2, 1])
                nc.tensor.matmul(psn[:], ones128[:], sqc_kc,
                                 start=True, stop=True)
                cn2 = spool.tile([1, K], F32, tag="cn2")
                cn2s = spool.tile([1, K], F32, tag="cn2s")
                nc.vector.tensor_reduce(cn2s[:], psn[:], mybir.AxisListType.X,
                                        AluOpType.add)
                nc.vector.tensor_scalar(cn2[:], cn2s[:], 0.25, None,
                                        AluOpType.mult)

                psg = psB.tile([LP, K], F32, tag="mm")
                for c in range(NCH):
                    nc.tensor.matmul(psg[:], dataT[:, c, :], ctrTs[:, c, :],
                                     start=(c == 0), stop=False)
                nc.tensor.matmul(psg[:], ones1[0:1, 0:LP], cn2[:],
                                 start=False, stop=True)
                nc.vector.tensor_scalar(d2_all[:, s, :], psg[:], xn2[:],
                                        None, AluOpType.add)
                if DEBUG:
                    nc.sync.dma_start(d2dbg[:, s, :], d2_all[:, s, :])
                    for c in range(NCH):
                        nc.sync.dma_start(ctdbg[:, c, :, s], ctrTs[:, c, :])

            # ---------------- batched decisions ----------------
            cd2 = ppool.tile([LP, S], F32)
            nc.vector.tensor_reduce(cd2[:], d2_all[:], mybir.AxisListType.X,
                                    AluOpType.min)

            lrelu = ppool.tile([LP, S], F32)
            nc.vector.tensor_scalar(lrelu[:], cd2[:], 0.0, None, AluOpType.max)
            lsq = ppool.tile([LP, S], F32)
            nc.scalar.activation(lsq[:], lrelu[:], AF_SQRT, bias=zeros98[:])
            psl = psS.tile([1, S], F32, tag="small")
            nc.tensor.matmul(psl[:], ones98[:], lsq[:], start=True, stop=True)
            lsum = ppool.tile([1, 1], F32)
            nc.vector.tensor_reduce(lsum[:], psl[:], mybir.AxisListType.X,
                                    AluOpType.add)
            nc.sync.dma_start(loss[:], lsum[:])

            m98 = ppool.tile([LP, 1], F32)
            nc.vector.tensor_reduce(m98[:], d2_all[:], mybir.AxisListType.XY,
                                    AluOpType.max)
            psmx = psT.tile([1, LP], F32, tag="pst")
            nc.tensor.transpose(psmx[:], m98[:], id128[0:LP, 0:LP])
            m98r = ppool.tile([1, LP], F32)
            nc.vector.tensor_copy(m98r[:], psmx[:])
            mx1 = ppool.tile([1, 1], F32)
            nc.vector.tensor_reduce(mx1[:], m98r[:], mybir.AxisListType.X,
                                    AluOpType.max)
            psm1 = psS.tile([LP, 1], F32, tag="small")
            nc.tensor.matmul(psm1[:], ones1[0:1, 0:LP], mx1[:],
                             start=True, stop=True)
            m1 = ppool.tile([LP, 1], F32)
            nc.vector.tensor_copy(m1[:], psm1[:])
            cds = ppool.tile([LP, S], F32)
            nc.vector.tensor_scalar(cds[:], cd2[:], m1[:, 0:1], 1.0,
                                    AluOpType.subtract, AluOpType.subtract)

            cd2b = _bc_free(cd2[:], 2, K)
            ikmb = _bc_free(ikm[:], 1, S)
            nc.vector.tensor_tensor(scr[:], d2_all[:], cd2b, AluOpType.is_equal)
            nc.vector.tensor_tensor(scr[:], scr[:], ikmb, AluOpType.mult)
            asg = ppool.tile([LP, S], F32)
            nc.vector.tensor_reduce(asg[:], scr[:], mybir.AxisListType.X,
                                    AluOpType.min)
            asgb = _bc_free(asg[:], 2, K)
            nc.vector.tensor_tensor(scr[:], asgb, ikmb, AluOpType.is_equal)
            cdsb = _bc_free(cds[:], 2, K)
            nc.vector.tensor_tensor(scr[:], scr[:], cdsb, AluOpType.mult)

            for s in range(S):
                pst2 = psT.tile([K, LP], F32, tag="pst")
                nc.tensor.transpose(pst2[:], scr[:, s, :], id128[0:LP, 0:LP])
                nc.vector.tensor_copy(valsT[:, s, :], pst2[:])

            minv = ppool.tile([K, S], F32)
            nc.vector.tensor_reduce(minv[:], valsT[:], mybir.AxisListType.X,
                                    AluOpType.min)
            minvb = _bc_free(minv[:], 2, LP)
            ilmb = _bc_free(ilm[:], 1, S)
            nc.vector.tensor_tensor(eqm[:], valsT[:], minvb, AluOpType.is_equal)
            nc.vector.tensor_tensor(eqm[:], eqm[:], ilmb, AluOpType.mult)
            mdm = ppool.tile([K, S], F32)
            nc.vector.tensor_reduce(mdm[:], eqm[:], mybir.AxisListType.X,
                                    AluOpType.min)

            # counting sort: C[n,v] = #{k: m_k <= v}; sorted[n,r] = #{v: C <= r}
            mdmb = _bc_free(mdm[:], 2, LP)
            nc.vector.tensor_tensor(cmpb[:], mdmb, ilmb, AluOpType.is_le)
            cmpb_f = cmpb[:].rearrange("k s l -> k (s l)")
            GRP = 5
            g0 = 0
            while g0 < S:
                cnt = min(GRP, S - g0)
                psc = psS.tile([1, GRP * LP], F32, tag="small")
                nc.tensor.matmul(psc[0:1, 0:cnt * LP], ones49b[:],
                                 cmpb_f[:, g0 * LP:(g0 + cnt) * LP],
                                 start=True, stop=True)
                nc.vector.tensor_copy(Csb[0:1, g0 * LP:(g0 + cnt) * LP],
                                      psc[0:1, 0:cnt * LP])
                g0 += cnt
            nc.sync.dma_start(csb_scr[:].unsqueeze(0), Csb[:])
            nc.sync.dma_start(C48[:], csb_scr[:].rearrange("(s v) -> s v", v=LP))

            C48b = _bc_free(C48[:], 1, K)
            irb = _bc_free(ir48[:], 2, LP)
            nc.vector.tensor_tensor(rk_t[:], C48b, irb, AluOpType.is_le)
            srt = ppool.tile([S, K], F32)
            nc.vector.tensor_reduce(srt[:], rk_t[:], mybir.AxisListType.X,
                                    AluOpType.add)

            # gather row indices into x viewed as [50*96, 768]:
            # row = 96*(1+l') + bt = 96 + 96*m - 4703*f + 2*s,  f = (m >= K)
            fge = ppool.tile([S, K], F32)
            nc.vector.tensor_scalar(fge[:], srt[:], float(K), None,
                                    AluOpType.is_ge)
            nc.vector.tensor_scalar(fge[:], fge[:], float(-(K * BT_C - 1)), None,
                                    AluOpType.mult)
            offs = ppool.tile([S, K], F32)
            nc.vector.scalar_tensor_tensor(offs[:], srt[:], float(BT_C), fge[:],
                                           AluOpType.mult, AluOpType.add)
            nc.vector.tensor_scalar(offs[:], offs[:], st48[:], None,
                                    AluOpType.add)
            psot = psT.tile([K, S], F32, tag="pst")
            nc.tensor.transpose(psot[:], offs[:], id128[0:S, 0:S])
            offsT = ppool.tile([K, S], F32)
            nc.vector.tensor_copy(offsT[:], psot[:])
            offsTu = ppool.tile([K, S], U32)
            nc.vector.tensor_copy(offsTu[:], offsT[:])

            xrows = x_ap.rearrange("l b w -> (l b) w")
            for s in range(S):
                gt = gpool.tile([K, W], F32, tag="gt")
                if STAGE >= 3:
                    nc.gpsimd.indirect_dma_start(
                        gt[:], None, xrows,
                        IndirectOffsetOnAxis(ap=offsTu[:, s:s + 1], axis=0))
                else:
                    nc.vector.memset(gt[:], 0.0)
                nc.sync.dma_start(out[1:L_TOK, s, :], gt[:])

    nc.compile()
    return nc


_CACHE = {}


def _get_nc():
    if "nc" not in _CACHE:
        _CACHE["nc"] = build()
    return _CACHE["nc"]


def kernel(**inputs):
    nc = _get_nc()
    x = np.ascontiguousarray(np.asarray(inputs["x"], dtype=np.float32))
    weights = {k: np.ascontiguousarray(np.asarray(v, dtype=np.float32))
               for k, v in inputs.items() if k != "x"}
    in_maps = []
    for c in range(NCORES):
        m = {"x": np.ascontiguousarray(x[:, c * BT_C:(c + 1) * BT_C, :])}
        m.update(weights)
        in_maps.append(m)
    res = run_bass_kernel_spmd(nc, in_maps, core_ids=list(range(NCORES)))
    out = np.concatenate([res.results[c]["out"] for c in range(NCORES)], axis=1)
    loss = np.float32(sum(float(res.results[c]["loss"][0, 0])
                          for c in range(NCORES)) / (B * AFTER_F))
    return out, loss
